# revision 19
# baseline (speedup 1.0000x reference)
"""PEER / product-key MoE routing kernel for Trainium2 (8 NeuronCores).

Strategy: data-parallel over tokens. Each of the 8 cores gets 256 of the
2048 tokens plus a full replica of the expert tables in its DRAM. Routing
(q projection, product-key scores, two-stage top-8), expert-row gathers,
and the PEER combine all run on-device. No collectives are needed; the
host only slices/packs inputs and concatenates the per-core outputs.

Per-core pipeline (v5):
  PE:  qT = Wq^T @ x^T with 128-wide feature tiles (fp32, exact);
       per-head scores via ONE matmul against block-diagonal keys.
  DVE: top-8 of each 256-score PSUM half via max8/max_index (exact),
       then top-8 of the 8x8 combo sums; winners' sub-key ids resolved
       with an is_equal one-hot reduction.
  PE+DVE: sub-key ids are shuffled into dma_gather's wrapped index
       layout ([16 partitions, n/16] int16) with 8 tiny permutation
       matmuls per token block (values <= 255, bf16-exact), then
       combined to expert ids and biased by -32768 (bitwise xor) so the
       17-bit id range fits int16 (the gather ucode uses a signed MAC;
       the source AP is pre-offset by +32768 rows to compensate).
  GPSIMD: ONE dma_gather per 8-slot group (1024 expert-row-pairs + 1
       pad index, 2 MB) - vectorized descriptor emission replaces 64
       per-slot indirect DMAs whose fixed costs starved the SDMA
       engines (the old path burned 150us of GpSimd time).
  DVE/ACT: inner products as bf16 multiply + free-dim-sum, split
       between the fused DVE scalar_tensor_tensor path and the
       DVE-mult + ACT-accum path to balance the two engines.
  PE:  combine as PSUM-accumulated diag(vals) @ w_up_row matmuls; the
       8 diag matrices of a group are built in one DVE op.

Routing is computed entirely in fp32, so expert selection matches the
fp32 reference exactly; only the expert tables are bf16 (rel err ~4e-3).
"""

import numpy as np

import concourse.bass as bass
import concourse.mybir as mybir
from concourse import bacc
from concourse import library_config
from concourse.bass import IndirectOffsetOnAxis
from concourse.tile import TileContext
from concourse.bass_utils import run_bass_kernel_spmd

N_CORES = 8
N_HEADS = 8
D_KEYS = 128
HALF = 64
N_KEYS = 256
TOP_K = 8
D = 512
B = 2048           # total tokens
BC = B // N_CORES  # tokens per core (256)
TB = BC // 128     # token blocks per core (2)
# dma_gather is limited to 1024 indices per op and trims trailing
# negative (biased) indices, so each op covers 7 real slot-chunks plus a
# pad chunk of constant index 0 (position 1023 >= 0 defeats the trim;
# mid-list negatives are fine - validity is position-based).
NOPS = 10          # gather ops per token block (9x 7 slots + 1x 1 slot)
NCOL = 64          # wrapped idx columns per op (1024/16)
F32 = mybir.dt.float32
U16 = mybir.dt.uint16
I16 = mybir.dt.int16
I32 = mybir.dt.int32
BF16 = mybir.dt.bfloat16
X = mybir.AxisListType.X
OP = mybir.AluOpType

# inner-product engine split within each 8-slot group:
# slots < STT_SPLIT use the fused DVE scalar_tensor_tensor, the rest use
# DVE-mult + ACT-accum (balances DVE vs Scalar engine load)
STT_SPLIT = 2


def build_nc(use_dg=True):
    nc = bacc.Bacc("TRN2", target_bir_lowering=False)

    xtokb_d = nc.dram_tensor("xtokb", [BC, D], BF16, kind="ExternalInput")
    xt_d = nc.dram_tensor("xt", [D, BC], F32, kind="ExternalInput")
    wq_d = nc.dram_tensor("wq", [D, N_HEADS * D_KEYS], F32, kind="ExternalInput")
    bqp_d = nc.dram_tensor("bqp", [D_KEYS, N_HEADS], F32, kind="ExternalInput")
    kbd_d = nc.dram_tensor("kbd", [D_KEYS, N_HEADS, 2, N_KEYS], F32,
                           kind="ExternalInput")
    wb_d = nc.dram_tensor("wb", [N_KEYS * N_KEYS, 2 * D], BF16,
                          kind="ExternalInput")
    id01_d = nc.dram_tensor("id01", [128, 128], BF16, kind="ExternalInput")
    shf_d = nc.dram_tensor("shf", [128, 8, 128], BF16, kind="ExternalInput")
    iota8_d = nc.dram_tensor("iota8", [128, 8], U16, kind="ExternalInput")
    out_d = nc.dram_tensor("out", [BC, D], F32, kind="ExternalOutput")

    with TileContext(nc) as tc:
        with (
            tc.tile_pool(name="const", bufs=1) as cpool,
            tc.tile_pool(name="psq", bufs=2, space="PSUM") as psq,
            tc.tile_pool(name="pss", bufs=2, space="PSUM") as pss,
            tc.tile_pool(name="pshf", bufs=2, space="PSUM") as pshfp,
            tc.tile_pool(name="st2", bufs=1) as st2,
            tc.tile_pool(name="eqs", bufs=2) as eqs,
            tc.tile_pool(name="wbp", bufs=4 if use_dg else 32) as wbp,
            tc.tile_pool(name="scr", bufs=4) as scrp,
            tc.tile_pool(name="dgp", bufs=4) as dgp,
            tc.tile_pool(name="pacc", bufs=1, space="PSUM") as paccp,
            tc.tile_pool(name="accp", bufs=2) as accp,
        ):
            if use_dg:
                nc.gpsimd.load_library(library_config.mlp)
            # ---- constant loads ----
            wq_sb = []
            xt_sb = []
            for k in range(4):
                t = cpool.tile([128, N_HEADS * D_KEYS], F32, tag=f"wq{k}")
                nc.sync.dma_start(out=t[:], in_=wq_d[k * 128:(k + 1) * 128, :])
                wq_sb.append(t)
                t2 = cpool.tile([128, BC], F32, tag=f"xt{k}")
                nc.sync.dma_start(out=t2[:], in_=xt_d[k * 128:(k + 1) * 128, :])
                xt_sb.append(t2)
            xtok_bf = []
            for tb in range(TB):
                tb16 = cpool.tile([128, D], BF16, tag=f"xtokb{tb}")
                nc.sync.dma_start(out=tb16[:], in_=xtokb_d[tb * 128:(tb + 1) * 128, :])
                xtok_bf.append(tb16)
            kbd_sb = cpool.tile([D_KEYS, N_HEADS, 2, N_KEYS], F32, tag="kbd")
            nc.sync.dma_start(out=kbd_sb[:], in_=kbd_d[:, :, :, :])
            id01_sb = cpool.tile([128, 128], BF16, tag="id01")
            nc.sync.dma_start(out=id01_sb[:], in_=id01_d[:, :])
            shf_sb = cpool.tile([128, 8, 128], BF16, tag="shf")
            nc.sync.dma_start(out=shf_sb[:], in_=shf_d[:, :, :])
            bqp_sb = cpool.tile([D_KEYS, N_HEADS], F32, tag="bqp")
            nc.sync.dma_start(out=bqp_sb[:], in_=bqp_d[:, :])
            iota8 = cpool.tile([128, 8], U16, tag="iota8")
            nc.sync.dma_start(out=iota8[:], in_=iota8_d[:, :])

            qts = [cpool.tile([D_KEYS, N_HEADS, 128], F32, tag=f"qt{tb}",
                              name=f"qt{tb}") for tb in range(TB)]
            widx = {}    # (tb) -> wrapped-index tile [128, 8, NCOL] U16
            idx32 = {}   # (tb, op) -> [128, gs] I32 (sim-fallback path)
            ws = {}
            pages = {}

            def routing(tb):
                tsl = slice(tb * 128, (tb + 1) * 128)
                qt = qts[tb]
                # ---- qT per head: [feature-in-head, token] (fp32, exact) ----
                for m in range(N_HEADS):
                    ps = psq.tile([128, 128], F32, tag="psq")
                    for k in range(4):
                        nc.tensor.matmul(
                            out=ps[:],
                            lhsT=wq_sb[k][:, m * 128:(m + 1) * 128],
                            rhs=xt_sb[k][:, tsl],
                            start=(k == 0),
                            stop=(k == 3),
                        )
                    nc.vector.tensor_scalar(
                        out=qt[:, m, :], in0=ps[:],
                        scalar1=bqp_sb[:, m:m + 1], scalar2=None, op0=OP.add,
                    )

                # ---- scores + stage-1 top8 (exact); one matmul per head ----
                s1t = st2.tile([128, 64], F32, tag=f"s1t{tb}", name=f"s1t{tb}")
                s2t = st2.tile([128, 64], F32, tag=f"s2t{tb}", name=f"s2t{tb}")
                i1 = st2.tile([128, 64], U16, tag=f"i1{tb}", name=f"i1{tb}")
                i2 = st2.tile([128, 64], U16, tag=f"i2{tb}", name=f"i2{tb}")
                for m in range(N_HEADS):
                    ps2 = pss.tile([128, 2, N_KEYS], F32, tag="pss")
                    nc.tensor.matmul(
                        out=ps2[:, :, :].rearrange("p a b -> p (a b)"),
                        lhsT=qt[:, m, :],
                        rhs=kbd_sb[:, m, :, :].rearrange("p a b -> p (a b)"),
                        start=True, stop=True,
                    )
                    for half, (st_, ix) in enumerate(((s1t, i1), (s2t, i2))):
                        nc.vector.max(out=st_[:, m * 8:(m + 1) * 8],
                                      in_=ps2[:, half, :])
                        nc.vector.max_index(
                            out=ix[:, m * 8:(m + 1) * 8],
                            in_max=st_[:, m * 8:(m + 1) * 8],
                            in_values=ps2[:, half, :],
                        )

                # ---- stage-2: 8x8 combo scores, top8 of 64 ----
                cs = st2.tile([128, 512], F32, tag=f"cs{tb}", name=f"cs{tb}")
                for m in range(N_HEADS):
                    nc.vector.tensor_tensor(
                        out=cs[:, m * 64:(m + 1) * 64].rearrange(
                            "p (a b) -> p a b", a=8),
                        in0=s1t[:, m * 8:(m + 1) * 8].unsqueeze(2).to_broadcast(
                            [128, 8, 8]),
                        in1=s2t[:, m * 8:(m + 1) * 8].unsqueeze(1).to_broadcast(
                            [128, 8, 8]),
                        op=OP.add,
                    )
                v8 = st2.tile([128, 64], F32, tag=f"v8{tb}", name=f"v8{tb}")
                n8 = st2.tile([128, 64], U16, tag=f"n8{tb}", name=f"n8{tb}")
                for m in range(N_HEADS):
                    nc.vector.max(out=v8[:, m * 8:(m + 1) * 8],
                                  in_=cs[:, m * 64:(m + 1) * 64])
                    nc.vector.max_index(
                        out=n8[:, m * 8:(m + 1) * 8],
                        in_max=v8[:, m * 8:(m + 1) * 8],
                        in_values=cs[:, m * 64:(m + 1) * 64])
                k1 = st2.tile([128, 64], U16, tag=f"k1{tb}", name=f"k1{tb}")
                nc.vector.tensor_scalar(
                    out=k1[:], in0=n8[:], scalar1=3, scalar2=None,
                    op0=OP.logical_shift_right)
                k2 = st2.tile([128, 64], U16, tag=f"k2{tb}", name=f"k2{tb}")
                nc.vector.tensor_scalar(
                    out=k2[:], in0=n8[:], scalar1=7, scalar2=None,
                    op0=OP.bitwise_and)

                # resolve winners' sub-key ids: isel[p,m,j] = i[p,m,k1[p,m,j]]
                sels = []
                for kk, ix in ((k1, i1), (k2, i2)):
                    eq = eqs.tile([128, 512], U16, tag="eq")
                    nc.vector.tensor_tensor(
                        out=eq[:, :].rearrange("p (m j k) -> p m j k", m=8, j=8),
                        in0=kk[:, :].rearrange("p (m j) -> p m j", m=8)
                            .unsqueeze(3).to_broadcast([128, 8, 8, 8]),
                        in1=iota8[:, :].unsqueeze(1).unsqueeze(1)
                            .to_broadcast([128, 8, 8, 8]),
                        op=OP.is_equal)
                    prod = eqs.tile([128, 512], U16, tag="prod")
                    nc.vector.tensor_tensor(
                        out=prod[:, :].rearrange("p (m j k) -> p m j k", m=8, j=8),
                        in0=eq[:, :].rearrange("p (m j k) -> p m j k", m=8, j=8),
                        in1=ix[:, :].rearrange("p (m k) -> p m k", m=8)
                            .unsqueeze(2).to_broadcast([128, 8, 8, 8]),
                        op=OP.mult)
                    sel = st2.tile([128, 64], U16, tag=f"sel{len(sels)}{tb}",
                                   name=f"sel{len(sels)}{tb}")
                    with nc.allow_low_precision(
                            reason="one-hot uint16 sum, values <= 255"):
                        nc.vector.reduce_sum(
                            out=sel[:],
                            in_=prod[:, :].rearrange("p (mj k) -> p mj k", k=8),
                            axis=X)
                    sels.append(sel)

                if use_dg:
                    # ---- wrap sub-key ids into dma_gather idx layout ----
                    # selb[:, 0:64] = sel1, [:, 64:128] = sel2 (bf16-exact)
                    selb = st2.tile([128, 128], BF16, tag=f"selb{tb}",
                                    name=f"selb{tb}")
                    nc.vector.tensor_copy(out=selb[:, 0:64], in_=sels[0][:])
                    nc.vector.tensor_copy(out=selb[:, 64:128], in_=sels[1][:])
                    w1 = st2.tile([128, NOPS, NCOL], U16, tag=f"w1{tb}",
                                  name=f"w1{tb}")
                    w2 = st2.tile([128, NOPS, NCOL], U16, tag=f"w2{tb}",
                                  name=f"w2{tb}")
                    for hi in range(8):
                        pf = pshfp.tile([128, 128], F32, tag="pshf")
                        nc.tensor.matmul(
                            out=pf[:], lhsT=shf_sb[:, hi, :], rhs=selb[:],
                            start=True, stop=True)
                        # W[q, o, j*8+hi] = sel[16*hi + q%16, slot 7o+j]
                        for w, base in ((w1, 0), (w2, 64)):
                            nc.vector.tensor_copy(
                                out=w[:, 0:9, hi:hi + 49:8],
                                in_=pf[:, base:base + 63].rearrange(
                                    "p (o j) -> p o j", o=9))
                            nc.vector.tensor_copy(
                                out=w[:, 9, hi:hi + 1],
                                in_=pf[:, base + 63:base + 64])
                    wx = st2.tile([128, NOPS, NCOL], U16, tag=f"wx{tb}",
                                  name=f"wx{tb}")
                    with nc.allow_low_precision(
                            reason="u16 expert-id packing, wraps by design"):
                        nc.vector.scalar_tensor_tensor(
                            out=wx[:], in0=w1[:], scalar=256, in1=w2[:],
                            op0=OP.mult, op1=OP.add)
                        nc.vector.tensor_scalar(
                            out=wx[:], in0=wx[:], scalar1=32768, scalar2=None,
                            op0=OP.bitwise_xor)
                        # pad chunks: idx 0 (>=0 as int16) so the last
                        # position of every op defeats the trailing trim
                        nc.vector.tensor_scalar(
                            out=wx[:, 0:9, 56:64], in0=wx[:, 0:9, 56:64],
                            scalar1=0, scalar2=None, op0=OP.mult)
                        nc.vector.tensor_scalar(
                            out=wx[:, 9, 8:16], in0=wx[:, 9, 8:16],
                            scalar1=0, scalar2=None, op0=OP.mult)
                    widx[tb] = wx
                else:
                    idx16 = st2.tile([128, 64], U16, tag=f"idx16{tb}",
                                     name=f"idx16{tb}")
                    nc.vector.tensor_scalar(
                        out=idx16[:], in0=sels[0][:], scalar1=256, scalar2=None,
                        op0=OP.mult)
                    nc.vector.tensor_tensor(
                        out=idx16[:], in0=idx16[:], in1=sels[1][:], op=OP.add)
                    for o in range(64):
                        ixg = st2.tile([128, 1], I32, tag=f"ixg{tb}_{o}",
                                       name=f"ixg{tb}_{o}")
                        nc.vector.tensor_copy(out=ixg[:],
                                              in_=idx16[:, o:o + 1])
                        idx32[(tb, o)] = ixg

                # ---- softmax over each head's top-8 ----
                rmax = st2.tile([128, 8], F32, tag=f"rmax{tb}", name=f"rmax{tb}")
                nc.vector.reduce_max(
                    out=rmax[:], in_=v8[:, :].rearrange("p (m k) -> p m k", m=8),
                    axis=X)
                ex = st2.tile([128, 64], F32, tag=f"ex{tb}", name=f"ex{tb}")
                nc.vector.tensor_tensor(
                    out=ex[:, :].rearrange("p (m k) -> p m k", m=8),
                    in0=v8[:, :].rearrange("p (m k) -> p m k", m=8),
                    in1=rmax[:, :].unsqueeze(2).to_broadcast([128, 8, 8]),
                    op=OP.subtract)
                nc.scalar.activation(
                    out=ex[:], in_=ex[:], func=mybir.ActivationFunctionType.Exp)
                rsum = st2.tile([128, 8], F32, tag=f"rsum{tb}", name=f"rsum{tb}")
                nc.vector.reduce_sum(
                    out=rsum[:], in_=ex[:, :].rearrange("p (m k) -> p m k", m=8),
                    axis=X)
                rinv = st2.tile([128, 8], F32, tag=f"rinv{tb}", name=f"rinv{tb}")
                nc.vector.reciprocal(out=rinv[:], in_=rsum[:])
                w8 = st2.tile([128, 64], F32, tag=f"w8{tb}", name=f"w8{tb}")
                nc.vector.tensor_tensor(
                    out=w8[:, :].rearrange("p (m k) -> p m k", m=8),
                    in0=ex[:, :].rearrange("p (m k) -> p m k", m=8),
                    in1=rinv[:, :].unsqueeze(2).to_broadcast([128, 8, 8]),
                    op=OP.mult)
                ws[tb] = w8

            def issue_gathers(tb):
                if use_dg:
                    for o in range(9):
                        page = wbp.tile([128, 8, 1024], BF16, tag="wbpage",
                                        name=f"pg{tb}_{o}")
                        pages[(tb, o)] = page
                        nc.gpsimd.dma_gather(
                            out_ap=page[:],
                            in_ap=wb_d[32768:, :],
                            idxs_ap=widx[tb][:, o, :].bitcast(I16),
                            num_idxs=897,
                            num_idxs_reg=897,
                            elem_size=1024,
                        )
                    page = wbp.tile([128, 2, 1024], BF16, tag="wbpage9",
                                    name=f"pg{tb}_9", bufs=2)
                    pages[(tb, 9)] = page
                    nc.gpsimd.dma_gather(
                        out_ap=page[:],
                        in_ap=wb_d[32768:, :],
                        idxs_ap=widx[tb][:, 9, 0:16].bitcast(I16),
                        num_idxs=129,
                        num_idxs_reg=129,
                        elem_size=1024,
                    )
                else:
                    for o in range(64):
                        page = wbp.tile([128, 1024], BF16, tag="wbpage",
                                        name=f"pg{tb}_{o}")
                        pages[(tb, o)] = page
                        nc.gpsimd.indirect_dma_start(
                            out=page[:], out_offset=None,
                            in_=wb_d[:, :],
                            in_offset=IndirectOffsetOnAxis(
                                ap=idx32[(tb, o)][:], axis=0),
                        )

            def get_page_slice(tb, grp, sidx, lo, hi):
                s = grp * 8 + sidx
                if use_dg:
                    o, j = (s // 7, s % 7) if s < 63 else (9, 0)
                    return pages[(tb, o)][:, j, lo:hi]
                return pages[(tb, s)][:, lo:hi]

            def compute(tb):
                inner = st2.tile([128, 64], F32, tag=f"inner{tb}",
                                 name=f"inner{tb}")
                va = st2.tile([128, 64], F32, tag=f"va{tb}", name=f"va{tb}")
                pacc = paccp.tile([128, D], F32, tag=f"pacc{tb}",
                                  name=f"pacc{tb}")
                for grp in range(8):
                    gs = slice(grp * 8, (grp + 1) * 8)
                    for sidx in range(8):
                        col = grp * 8 + sidx
                        wdrow = get_page_slice(tb, grp, sidx, 0, D)
                        scr = scrp.tile([128, D], BF16, tag="scr")
                        if sidx < STT_SPLIT:
                            nc.vector.scalar_tensor_tensor(
                                out=scr[:], in0=wdrow, scalar=1.0,
                                in1=xtok_bf[tb][:], op0=OP.mult, op1=OP.mult,
                                accum_out=inner[:, col:col + 1])
                        else:
                            nc.vector.tensor_tensor(
                                out=scr[:], in0=wdrow,
                                in1=xtok_bf[tb][:], op=OP.mult)
                            scr2 = scrp.tile([128, D], BF16, tag="scr2")
                            nc.scalar.activation(
                                out=scr2[:], in_=scr[:],
                                func=mybir.ActivationFunctionType.Copy,
                                accum_out=inner[:, col:col + 1])
                    rl8 = st2.tile([128, 8], F32, tag=f"rl{tb}", name=f"rl{tb}")
                    nc.scalar.activation(
                        out=rl8[:], in_=inner[:, gs],
                        func=mybir.ActivationFunctionType.Relu)
                    nc.vector.tensor_tensor(
                        out=va[:, gs], in0=rl8[:], in1=ws[tb][:, gs],
                        op=OP.mult)
                    # all 8 diag matrices of the group in one DVE op
                    dgrp = dgp.tile([128, 8, 128], BF16, tag="dgrp")
                    nc.vector.tensor_tensor(
                        out=dgrp[:],
                        in0=va[:, gs].unsqueeze(2).to_broadcast([128, 8, 128]),
                        in1=id01_sb[:].unsqueeze(1).to_broadcast([128, 8, 128]),
                        op=OP.mult)
                    for sidx in range(8):
                        col = grp * 8 + sidx
                        nc.tensor.matmul(
                            out=pacc[:], lhsT=dgrp[:, sidx, :],
                            rhs=get_page_slice(tb, grp, sidx, D, 2 * D),
                            start=(col == 0), stop=(col == 63))
                acc_sb = accp.tile([128, D], F32, tag=f"acc{tb}",
                                   name=f"acc{tb}")
                nc.vector.tensor_copy(out=acc_sb[:], in_=pacc[:])
                nc.sync.dma_start(
                    out=out_d[tb * 128:(tb + 1) * 128, :], in_=acc_sb[:])

            for tb in range(TB):
                routing(tb)
                issue_gathers(tb)
            for tb in range(TB):
                compute(tb)

    nc.compile()
    return nc


_NC_CACHE = None
VARIANT = {}


def _get_nc():
    global _NC_CACHE
    if _NC_CACHE is None:
        _NC_CACHE = build_nc(**VARIANT)
    return _NC_CACHE


def _prep_in_maps(inputs):
    q = np.ascontiguousarray(np.asarray(inputs["queries"], dtype=np.float32))
    Wq = np.ascontiguousarray(np.asarray(inputs["Wq"], dtype=np.float32))
    bq = np.asarray(inputs["bq"], dtype=np.float32)
    keys = np.asarray(inputs["keys"], dtype=np.float32)
    wd = np.asarray(inputs["w_down"], dtype=np.float32)
    wu = np.asarray(inputs["w_up"], dtype=np.float32)
    import ml_dtypes
    wb = np.ascontiguousarray(
        np.concatenate([wd, wu], axis=1).astype(ml_dtypes.bfloat16))
    id01 = np.eye(128, dtype=np.float32).astype(ml_dtypes.bfloat16)

    x = q.reshape(B, D)
    # bqp[p, m] = bq[m*128 + p]
    bqp = np.ascontiguousarray(bq.reshape(N_HEADS, D_KEYS).T)
    # block-diagonal keys: kbd[p, m, 0, n] = keys[m, 0, n, p] for p < 64,
    # kbd[p, m, 1, n] = keys[m, 1, n, p-64] for p >= 64, zero elsewhere.
    kbd = np.zeros((D_KEYS, N_HEADS, 2, N_KEYS), np.float32)
    kbd[:HALF, :, 0, :] = keys[:, 0].transpose(2, 0, 1)
    kbd[HALF:, :, 1, :] = keys[:, 1].transpose(2, 0, 1)
    kbd = np.ascontiguousarray(kbd)
    # shuffle/replicate matrices: shf[p, hi, f] = 1 iff p == 16*hi + f%16
    shf = np.zeros((128, 8, 128), np.float32)
    for hi in range(8):
        for f in range(128):
            shf[16 * hi + f % 16, hi, f] = 1.0
    shf = np.ascontiguousarray(shf.astype(ml_dtypes.bfloat16))
    iota8 = np.broadcast_to(np.arange(8, dtype=np.uint16), (128, 8))
    iota8 = np.ascontiguousarray(iota8)

    in_maps = []
    for c in range(N_CORES):
        xc = x[c * BC:(c + 1) * BC]
        in_maps.append({
            "xtokb": np.ascontiguousarray(xc.astype(ml_dtypes.bfloat16)),
            "xt": np.ascontiguousarray(xc.T),
            "wq": Wq,
            "bqp": bqp,
            "kbd": kbd,
            "wb": wb,
            "id01": id01,
            "shf": shf,
            "iota8": iota8,
        })
    return in_maps


def run(inputs, trace=False):
    """Run on 8 NeuronCores; returns (out [2,1024,512], BassKernelResults)."""
    nc = _get_nc()
    in_maps = _prep_in_maps(inputs)
    res = run_bass_kernel_spmd(
        nc, in_maps, core_ids=list(range(N_CORES)), trace=trace)
    out = np.concatenate(
        [res.results[c]["out"] for c in range(N_CORES)], axis=0)
    return out.reshape(2, 1024, D), res


def kernel(**inputs) -> np.ndarray:
    out, _ = run(inputs, trace=False)
    return out
